# revision 2
# baseline (speedup 1.0000x reference)
"""EnhancedBalSCL TRN2 kernel, v2: weight-folded exp + ACT-accum row sums.

Same symmetric block decomposition as v1 (see kernel.py docstring): core c
computes its diagonal 512x512 block, off-diagonal blocks vs cores c+1..c+3
(and c+4 for c<4; zero-padded dummy otherwise), and the 1000 class centers;
off-diagonal blocks also yield PE col-sums exchanged through the host.

v2 folds the per-column class weights INTO the matmul: feature dims 1022 and
1023 of the fp8 operands are repurposed as a bias pair -- the lhs (stationary)
rows are 1.0 there, and each rhs column k carries a residual-encoded
fp8(b1)+fp8(b2) ~= ln(w_k)/10.  The raw block then satisfies
  exp(10*raw'[i,k]) = w'_k * exp(10*raw[i,k]),   w'_k = exp(10*(b1+b2)) ~ w_k
so the ACT exp instruction's accum_out produces the weighted row-sums
directly and the DVE weighted-sum stage (and its wv operand) disappears.
The host knows w' exactly and divides the exchanged col-sums by it; dummy and
pad columns carry bias -240 so their exp underflows to exactly 0.

Other v2 deltas vs v1:
  - col-sum PSUM target merged into one bank (partitions 0/32/64/96 of a
    single [128,512] tile, bufs=2) freeing a PSUM bank,
  - cs -> SBUF staging copies moved to the (now idle) DVE,
  - center columns trimmed 1024 -> 1000 (pads never computed),
  - the reps-loop (timing builds only) uses staggered_reset so iterations
    overlap instead of paying a ~3us all-engine barrier.

Device outputs per core:
  out  [128, 16] f32: ACT-accum row-sum partials, col 4m+u as in v1
  out2 [4, 512]  f32: col-sums of the 4 off-diag blocks (q -> block c+1+q)
"""

import numpy as np
import ml_dtypes

_B, _D, _C, _M = 4096, 1024, 1000, 8
_BL = _B // _M            # 512 rows per core
_RT = _BL // 128          # 4 row tiles per core
_JT = _D // 256           # 4 super-K tiles (fp8 DoubleRow path)
_XC = 2048                # off-diagonal rhs columns per core (4 blocks)
_CP = 1024                # padded class dim (storage; only 1000 computed)
_CW = 1000                # computed center columns
_SCALE = 10.0             # 1/tau
_BD = _D - 2              # feature dims actually carrying data (bias pair)
_NEG = -240.0             # bias value that underflows exp to 0

_CACHE = {}


def _build_nc(reps=1, staggered=True):
    import concourse.bass as bass
    import concourse.mybir as mybir
    from concourse import bacc, tile
    from contextlib import ExitStack

    f32 = mybir.dt.float32
    bf16 = mybir.dt.bfloat16
    fp8 = mybir.dt.float8e4
    DR = mybir.MatmulPerfMode.DoubleRow
    AF = mybir.ActivationFunctionType
    OP = mybir.AluOpType

    nc = bacc.Bacc("TRN2", target_bir_lowering=False, debug=False,
                   num_devices=_M)
    fh_d = nc.declare_dram_parameter("flh", [_JT, 2, 128, _BL], fp8, isOutput=False)
    l8_d = nc.declare_dram_parameter("fl8", [_JT, 2, 128, _BL], fp8, isOutput=False)
    fs_d = nc.declare_dram_parameter("fts", [_JT, 2, 128, _XC], fp8, isOutput=False)
    rc_d = nc.declare_dram_parameter("rc8", [_JT, 2, 128, _CP], fp8, isOutput=False)
    wc_d = nc.declare_dram_parameter("wcol", [128, _RT], bf16, isOutput=False)
    out_d = nc.declare_dram_parameter("out", [128, 16], f32, isOutput=True)
    o2_d = nc.declare_dram_parameter("out2", [4, 512], f32, isOutput=True)

    with tile.TileContext(nc) as tc, ExitStack() as ctx:
        consts = ctx.enter_context(tc.tile_pool(name="consts", bufs=1))
        psum = ctx.enter_context(tc.tile_pool(name="psum", bufs=1, space="PSUM"))
        xps = ctx.enter_context(tc.tile_pool(name="xps", bufs=4))

        flh = consts.tile([128, _JT * 2 * _BL], fp8, tag="flh", name="flh")
        fl8 = consts.tile([128, _JT * 2 * _BL], fp8, tag="fl8", name="fl8")
        fts = consts.tile([128, _JT * 2 * _XC], fp8, tag="fts", name="fts")
        rc8t = consts.tile([128, _JT * 2 * _CP], fp8, tag="rct8", name="rc8t")
        wcol = consts.tile([128, _RT], bf16, tag="wcol", name="wcol")
        outt = consts.tile([128, 16], f32, tag="outt", name="outt")
        o2s = consts.tile([128, 512], f32, tag="o2s", name="o2s")

        def chunk(dst, src, j, eng, cols):
            eng.dma_start(
                dst[:, j * 2 * cols:(j + 1) * 2 * cols].rearrange(
                    "p (i c) -> p i c", i=2),
                src[j].rearrange("i p c -> p i c"))

        def fts_chunk(j, g, eng):
            eng.dma_start(
                fts[:, j * 2 * _XC:(j + 1) * 2 * _XC]
                .rearrange("p (i c) -> p i c", i=2)[:, :, g * 1024:(g + 1) * 1024],
                fs_d[j, :, :, g * 1024:(g + 1) * 1024].rearrange("i p c -> p i c"))

        # sync queue: the diag gate (flh+fl8 j0/j1), then fts in use order
        for j in (0, 1):
            chunk(flh, fh_d, j, nc.sync, _BL)
            chunk(fl8, l8_d, j, nc.sync, _BL)
        for j in range(_JT):
            fts_chunk(j, 0, nc.sync)
        for j in range(_JT):
            fts_chunk(j, 1, nc.sync)
        # gpsimd queue: flh+fl8 j2/j3, wcol, centers
        for j in (2, 3):
            chunk(flh, fh_d, j, nc.gpsimd, _BL)
            chunk(fl8, l8_d, j, nc.gpsimd, _BL)
        nc.gpsimd.dma_start(wcol[:], wc_d[:])
        for j in range(_JT):
            chunk(rc8t, rc_d, j, nc.gpsimd, _CP)

        lhs8 = [[flh[:, j * 2 * _BL:(j + 1) * 2 * _BL]
                 .rearrange("p (i c) -> p i c", i=2)[:, :, m * 128:(m + 1) * 128]
                 for j in range(_JT)] for m in range(_RT)]

        def blk():
            return psum.tile([128, 1024], f32, tag="blk", bufs=3, name="ps")

        def mm_cols(ps, m, src_tile, src_w, lo, width):
            # raw block [128, width] from columns lo:lo+width of src_tile
            for j in range(_JT):
                rj = src_tile[:, j * 2 * src_w:(j + 1) * 2 * src_w].rearrange(
                    "p (i c) -> p i c", i=2)
                o = 0
                while o < width:
                    wdt = min(512, width - o)
                    nc.tensor.matmul(ps[:, o:o + wdt], lhs8[m][j],
                                     rj[:, :, lo + o:lo + o + wdt],
                                     start=(j == 0), stop=(j == _JT - 1),
                                     perf_mode=DR)
                    o += wdt

        def esum(ps, width, acc_col):
            # exp with the folded column weight; accum_out = weighted row-sums
            xp = xps.tile([128, 1024], bf16, tag="xp", bufs=4, name="xp")
            nc.scalar.activation(xp[:, :width], ps[:, :width], AF.Exp,
                                 scale=_SCALE,
                                 accum_out=outt[:, acc_col:acc_col + 1])
            return xp

        def body(_i=None):
            cs = psum.tile([128, 512], f32, tag="cs", bufs=2, name="cs")

            deferred = []

            def drain(keep):
                # colsum matmuls lag their block so PE never waits on the
                # ACT->SBUF exp ack latency
                while len(deferred) > keep:
                    deferred.pop(0)()

            def colsum_of(pair, m, xp):
                def emit():
                    for h in (0, 1):
                        q = 2 * pair + h
                        nc.tensor.matmul(
                            cs[32 * q:32 * q + 1, 0:512], wcol[:, m:m + 1],
                            xp[:, h * 512:h * 512 + 512],
                            start=(m == 0), stop=(m == _RT - 1),
                            tile_position=(0, 32 * q))
                return emit

            # diag units m0..m2 start the pipeline (only flh/fl8 needed)
            for m in range(_RT - 1):
                ps = blk()
                mm_cols(ps, m, fl8, _BL, 0, 512)
                esum(ps, 512, 4 * m)
            # off-diagonal pair units; col-sums accumulate in cs, lagged 2
            for pair in (0, 1):
                for m in range(_RT):
                    ps = blk()
                    mm_cols(ps, m, fts, _XC, pair * 1024, 1024)
                    drain(2)
                    xp = esum(ps, 1024, 4 * m + 1 + pair)
                    deferred.append(colsum_of(pair, m, xp))
            # centers (only 1000 real columns computed)
            for m in range(_RT):
                ps = blk()
                mm_cols(ps, m, rc8t, _CP, 0, _CW)
                drain(1 - m if m < 2 else 0)
                esum(ps, _CW, 4 * m + 3)
            drain(0)
            # stage the finished col-sums to SBUF on the idle DVE
            for q in range(4):
                nc.vector.tensor_scalar_add(o2s[32 * q:32 * q + 1, :],
                                            cs[32 * q:32 * q + 1, :], 0.0)
            # diag m3: the short final block
            ps = blk()
            mm_cols(ps, 3, fl8, _BL, 0, 512)
            esum(ps, 512, 12)

        if reps == 1:
            body()
        else:
            with tc.For_i(0, reps, 1, staggered_reset=staggered,
                          hint_engines=(mybir.EngineType.PE,)) as i:
                body(i)

        nc.gpsimd.dma_start(
            o2_d[0:4],
            o2s[:].rearrange("(a b) c -> a b c", b=32)[:, 0:1, :])
        nc.sync.dma_start(out_d[:], outt[:])

    nc.compile()
    return nc


def _get_nc():
    if "nc" not in _CACHE:
        _CACHE["nc"] = _build_nc()
    return _CACHE["nc"]


def _enc_bias(lnw10):
    """Residual fp8 encoding of ln(w)/10: returns (b1, b2, w_prime)."""
    fp8 = ml_dtypes.float8_e4m3
    b = np.asarray(lnw10, dtype=np.float32)
    b1 = b.astype(fp8)
    b2 = (b - b1.astype(np.float32)).astype(fp8)
    wp = np.exp(np.float32(10.0) *
                (b1.astype(np.float32) + b2.astype(np.float32)))
    return b1, b2, wp.astype(np.float32)


def _prep_inputs(centers, features, targets):
    bf16 = ml_dtypes.bfloat16
    fp8 = ml_dtypes.float8_e4m3
    F = np.ascontiguousarray(features, dtype=np.float32)      # [B, D]
    Cen = np.ascontiguousarray(centers, dtype=np.float32)     # [C, D]
    t = np.asarray(targets).astype(np.int64).ravel()          # [B]

    counts = np.bincount(t, minlength=_C).astype(np.float32)
    w = (1.0 / (counts[t] + 1.0)).astype(np.float32)
    v = (1.0 / (counts + 1.0)).astype(np.float32)
    H = np.zeros((_C, _D), dtype=np.float32)
    np.add.at(H, t, F)

    F8 = F.astype(fp8)
    F8f = F8.astype(np.float32)

    # lhs: bias dims carry 1.0
    Flh = F8.copy()
    Flh[:, _BD:] = np.float32(1.0)
    FlhT = np.ascontiguousarray(Flh.T)                        # [D, B] fp8

    # rhs batch columns: bias dims carry the residual-encoded ln(w)/10
    b1, b2, wprime = _enc_bias(np.log(w.astype(np.float64)) / 10.0)
    Frh = F8.copy()
    Frh[:, _BD] = b1
    Frh[:, _BD + 1] = b2
    FrhT = np.ascontiguousarray(Frh.T)                        # [D, B] fp8

    # rhs center columns
    c1, c2, vprime = _enc_bias(np.log(v.astype(np.float64)) / 10.0)
    CT8 = np.full((_D, _CP), np.float32(_NEG), dtype=fp8)
    CT8[:, :_C] = Cen.astype(fp8).T
    CT8[_BD, :_C] = c1
    CT8[_BD + 1, :_C] = c2
    rc8 = np.ascontiguousarray(CT8.reshape(_JT, 2, 128, _CP))

    wb = w.astype(bf16)

    # host-side finals: positive term P and the diag-term cancellation.
    # P approximates the reference positives from the (unmodified) fp8 data.
    U8f = (H + Cen).astype(fp8).astype(np.float32)
    P = np.einsum("bd,bd->b", F8f, U8f[t, :], dtype=np.float32)
    diag8 = np.einsum("bd,bd->b", F8f, F8f, dtype=np.float32)
    pos = (P - diag8) * (np.float32(_SCALE) / counts[t])

    # device self term mirror: ACT exp accumulates pre-rounding f32 values of
    # exp(10*(q8' + b1 + b2)) where q8' drops the two bias dims
    q8p = np.einsum("bd,bd->b", F8f[:, :_BD], F8f[:, :_BD], dtype=np.float32)
    dev_diag = wprime * np.exp(np.float32(_SCALE) * q8p)
    q = (F * F).sum(axis=1)
    corr = w * np.exp(np.float32(_SCALE) * q) - dev_diag

    def col(x_loc):
        return np.ascontiguousarray(x_loc.reshape(_RT, 128).T)

    in_maps = []
    host = []
    for c in range(_M):
        R = c * _BL
        flh = np.ascontiguousarray(FlhT[:, R:R + _BL]).reshape(_JT, 2, 128, _BL)
        fl8 = np.ascontiguousarray(FrhT[:, R:R + _BL]).reshape(_JT, 2, 128, _BL)
        # off-diagonal rhs: blocks c+1, c+2, c+3 and (c<4) c+4, else dummy
        blocks = [(c + 1) % _M, (c + 2) % _M, (c + 3) % _M]
        if c < 4:
            blocks.append(c + 4)
        cols = np.concatenate([np.arange(b * _BL, (b + 1) * _BL) for b in blocks])
        ftsb = np.zeros((_D, _XC), dtype=fp8)
        ftsb[_BD:, :] = np.float32(_NEG)  # dummy cols: exp underflows to 0
        ftsb[:, :len(cols)] = FrhT[:, cols]
        ftsb = np.ascontiguousarray(ftsb.reshape(_JT, 2, 128, _XC))
        in_maps.append({
            "flh": flh, "fl8": fl8, "fts": ftsb, "rc8": rc8,
            "wcol": col(wb[R:R + _BL].astype(np.float32)).astype(bf16),
        })
        host.append({
            "corr": col(corr[R:R + _BL]),
            "pos": col(pos[R:R + _BL]),
            "blocks": blocks,
        })
    _CACHE["host"] = host
    _CACHE["wprime"] = wprime
    return in_maps


def _assemble(results):
    host = _CACHE["host"]
    wprime = _CACHE["wprime"]
    # scatter col-sum contributions; they carry the target column's folded
    # weight w'_k, which the row-sum convention does not want -- divide out
    den_col = np.zeros(_B, dtype=np.float32)
    for c in range(_M):
        o2 = np.asarray(results[c]["out2"], dtype=np.float32)
        for qi, b in enumerate(host[c]["blocks"]):
            den_col[b * _BL:(b + 1) * _BL] += o2[qi]
    den_col = den_col / wprime
    total = 0.0
    for c in range(_M):
        o = np.asarray(results[c]["out"], dtype=np.float32)
        h = host[c]
        den = o[:, 0::4] + o[:, 1::4] + o[:, 2::4] + o[:, 3::4]  # [128, RT]
        R = c * _BL
        den = den + den_col[R:R + _BL].reshape(_RT, 128).T + h["corr"]
        per = np.log(den) - h["pos"]
        total += float(per.sum())
    return np.float32(total / _B)


def _run(inputs, trace=False, **trace_kwargs):
    from concourse.bass_utils import run_bass_kernel_spmd
    nc = _get_nc()
    in_maps = _prep_inputs(**inputs)
    res = run_bass_kernel_spmd(nc, in_maps, core_ids=list(range(_M)),
                               trace=trace, **trace_kwargs)
    return _assemble(res.results), res


def kernel(centers, features, targets):
    out, _ = _run({"centers": centers, "features": features, "targets": targets})
    return out


_M = _M  # re-export for harnesses
